# revision 13
# baseline (speedup 1.0000x reference)
"""LocallyConnected2d (untied-weights conv) Trainium2 kernel.

Math: out[b, i, j] = sum_{kh, kw} x[b, i+kh, j+kw] * K[i, j, kh, kw] + bias[i, j]
with B=64, input 128x128, taps 8x8, output 121x121, fp32.

Strategy (output rows sharded across 8 cores, full batch per core):
  Per-output-position weights rule out weight-stationary matmul, but for a
  fixed output row i and tap row kh the contribution
    C[b, j] = sum_kw K[i, j, kh, kw] * x[b, i+kh, j+kw]
  is a matmul over the full input row (contraction w = j + kw):
    C[b, j] = sum_w xT_r[w, b] * Band[w, j],  r = i + kh
  where Band[w, c=j+7] = K[i, j, kh, w-j] for w-j in [0,8) — a banded
  128x128 matrix. PSUM accumulates the 8 kh contributions per output row
  (plus a K=1 ones-matmul that injects the bias), so each output row costs
  8 matmuls of N=121 columns on the TensorEngine.

  Band tiles are materialized with "skew" DMAs: a strided DVE copy first
  rearranges the kernel slab into anti-diagonal 8-element runs (RA), then a
  per-tile SBUF->SBUF DMA with a diagonal access pattern (partition step
  row+1) writes run w to (partition w, cols w..w+7). The DMA engines only
  decompose such diagonals correctly for 512-byte rows, so band tiles are
  standalone [128, 128] fp32 tensors; a second small rectangular DMA fills
  the 7x7 corner (w>120) from a separately prepared RB buffer. Kernel taps
  are staged in 16-wide slots (8 real + 8 zero) so out-of-range tap reads
  land on zeros.

  xT (x rows transposed to [w, b]) is produced on-chip by PE transposes.
"""

import sys

import numpy as np

try:
    import concourse.bacc as bacc
except ImportError:
    sys.path.insert(0, "/opt/trn_rl_repo")
    import concourse.bacc as bacc

import concourse.mybir as mybir
from concourse import tile
from concourse.bass_utils import run_bass_kernel_spmd
from concourse.masks import make_identity

F32 = mybir.dt.float32

B = 64
IN_H = IN_W = 128
KH = KW = 8
OH = OW = 121
NCORES = 8
NI = 16              # output rows computed per core
NR = NI + KH - 1     # input rows needed per core (23)
NT = NI * KH         # band tiles per core (128)
I0 = [0, 16, 32, 48, 64, 80, 96, 105]  # first output row per core (core 7 overlaps)

SLOT = 16                      # kw slots per j (8 real + 8 zero pad)
KT_COLS = OW + KH + 6          # 135 j-slots (j' = j + 7, zero pad both ends)
KT_ROW = KT_COLS * SLOT        # 2160 elements per partition
RA_ROW = 128 * KW              # 1024
RB_ROW = 49
NBUF = 32                      # band-tile ring depth

_nc_cache = []
TRACE = False          # set by test.py to capture an NTFF profile
LAST_RESULTS = None    # BassKernelResults of the most recent run


def _build_nc():
    nc = bacc.Bacc(detect_race_conditions=False)
    x_in = nc.dram_tensor("xs", [B, NR, IN_W], F32, kind="ExternalInput")
    k_in = nc.dram_tensor("ks", [NI, OW, KH, KW], F32, kind="ExternalInput")
    b_in = nc.dram_tensor("bs", [NI, OW], F32, kind="ExternalInput")
    o_out = nc.dram_tensor("os", [B, NI, OW], F32, kind="ExternalOutput")

    with tile.TileContext(nc) as tc:
        with (
            tc.tile_pool(name="sb", bufs=1) as sb,
            tc.tile_pool(name="ps", bufs=1, space="PSUM") as ps,
        ):
            XS = sb.tile([B, NR * IN_W], F32, tag="xs")      # x slab, b on partitions
            XT = sb.tile([128, NR * B], F32, tag="xt")       # transposed rows [w, r*64+b]
            KT = sb.tile([128, KT_ROW], F32, tag="kt")       # taps, part=(ig,kh), 16-wide slots
            RA = sb.tile([128, RA_ROW], F32, tag="ra")       # anti-diagonal runs
            RB = sb.tile([128, RB_ROW], F32, tag="rb")       # 7x7 corner values
            OS = sb.tile([B, NI * OW], F32, tag="os")        # output staging
            BIAS = sb.tile([1, NI * OW], F32, tag="bias")
            ONES = sb.tile([1, B], F32, tag="ones")
            ID64 = sb.tile([B, B], F32, tag="id64")
            RING = [
                sb.tile([128, 128], F32, tag=f"bt{s}", name=f"bt{s}")
                for s in range(NBUF)
            ]

            PSB = [
                ps.tile([B, 4 * OW], F32, tag=f"psb{g}", name=f"psb{g}")
                for g in range(4)
            ]

            # ---- input DMAs ----
            nc.sync.dma_start(out=XS[:, :], in_=x_in[:, :, :])
            nc.sync.dma_start(out=BIAS[0:1, :], in_=b_in[:, :])
            # zero KT (covers j pads and kw slots 8..15), then load taps
            nc.vector.memset(KT[:, 0 : KT_ROW // 2], 0.0)
            nc.gpsimd.memset(KT[:, KT_ROW // 2 : KT_ROW], 0.0)
            for ig in range(NI):
                kt_dst = KT[ig * KH : ig * KH + 1, 7 * SLOT : 7 * SLOT + 1].copy()
                kt_dst.ap[:] = [[KT_ROW, KH], [SLOT, OW], [1, KW]]
                kt_src = k_in[ig].copy()  # [121j, 8kh, 8kw] -> dims (kh, j, kw)
                kt_src.ap[:] = [[KW, KH], [KH * KW, OW], [1, KW]]
                nc.sync.dma_start(out=kt_dst, in_=kt_src)

            # ---- constants ----
            make_identity(nc, ID64[:, :])
            nc.gpsimd.memset(ONES[0:1, :], 1.0)

            # ---- zero the band ring (one-time; skew rewrites land on the
            # same cells every reuse, so zeros stay valid) ----
            for s in range(NBUF):
                eng = nc.vector if s % 2 == 0 else nc.gpsimd
                eng.memset(RING[s][:, :], 0.0)

            # ---- RA: RA[p][w*8+q] = KT[p][(w+q)*16 + (7-q)] ----
            ra_dst = RA[:, :].rearrange("p (w q) -> p w q", q=KW)
            ra_src = KT[:, 0:1].copy()
            ra_src.ap[:] = [[KT_ROW, 128], [SLOT, 128], [SLOT - 1, KW]]
            ra_src2 = KT[:, 7 : 7 + 1].copy()
            ra_src2.ap[:] = [[KT_ROW, 128], [SLOT, 128], [SLOT - 1, KW]]
            nc.vector.tensor_copy(out=ra_dst, in_=ra_src2)

            # ---- RB: RB[p][w'*7+c'] = KT[p][(c'+121)*16 + (w'-c'+7)] ----
            rb_dst = RB[:, :].rearrange("p (w c) -> p w c", c=7)
            rb_src = KT[:, 1943 : 1943 + 1].copy()
            rb_src.ap[:] = [[KT_ROW, 128], [1, 7], [SLOT - 1, 7]]
            nc.vector.tensor_copy(out=rb_dst, in_=rb_src)

            # ---- skew DMAs: RA/RB -> band tiles ----
            for t in range(NT):
                bt = RING[t % NBUF]
                a_src = RA[t : t + 1, :].copy()
                a_src.ap[:] = [[RA_ROW, 1], [KW, OW], [1, KW]]
                a_dst = bt[0:1, 0:1].copy()
                a_dst.ap[:] = [[129, OW], [1, KW]]
                nc.sync.dma_start(out=a_dst, in_=a_src)
                b_src = RB[t : t + 1, :].copy()
                b_src.ap[:] = [[RB_ROW, 1], [7, 7], [1, 7]]
                b_dst = bt[121:122, 121:122].copy()
                b_dst.ap[:] = [[128, 7], [1, 7]]
                nc.scalar.dma_start(out=b_dst, in_=b_src)

            # ---- PE transposes: x rows -> XT[w, r*64+b] ----
            for r in range(NR):
                tp = ps.tile([128, B], F32, tag="tp", bufs=3, name=f"tp{r}")
                nc.tensor.transpose(
                    tp[:, :], XS[:, r * IN_W : (r + 1) * IN_W], ID64[:, :]
                )
                nc.scalar.copy(out=XT[:, r * B : (r + 1) * B], in_=tp[:, :])

            # ---- bias init (K=1 ones-matmul, whole bank, starts the group) ----
            for g in range(4):
                nc.tensor.matmul(
                    PSB[g][0:B, :],
                    ONES[0:1, :],
                    BIAS[0:1, g * 4 * OW : (g + 1) * 4 * OW],
                    start=True,
                    stop=False,
                    skip_group_check=True,
                )

            # ---- main banded matmuls ----
            for rl in range(NR):
                lhsT = XT[:, rl * B : (rl + 1) * B]
                for kh in range(KH):
                    il = rl - kh
                    if 0 <= il < NI:
                        t = il * KH + kh
                        g, s = il // 4, il % 4
                        nc.tensor.matmul(
                            PSB[g][0:B, s * OW : (s + 1) * OW],
                            lhsT,
                            RING[t % NBUF][:, 7:128],
                            start=False,
                            stop=(s == 3 and kh == KH - 1),
                            skip_group_check=True,
                        )
                # retire a full psum bank once its 4 output rows are done
                for g in range(4):
                    if rl == 4 * g + 3 + KH - 1:
                        nc.scalar.copy(
                            out=OS[:, g * 4 * OW : (g + 1) * 4 * OW],
                            in_=PSB[g][0:B, :],
                        )

            nc.sync.dma_start(out=o_out[:, :, :], in_=OS[:, :])

    nc.finalize()
    return nc


def kernel(x, kernels, bias):
    x = np.ascontiguousarray(x, dtype=np.float32)
    kernels = np.ascontiguousarray(kernels, dtype=np.float32)
    bias = np.ascontiguousarray(bias, dtype=np.float32)

    if not _nc_cache:
        _nc_cache.append(_build_nc())
    nc = _nc_cache[0]

    in_maps = []
    for c in range(NCORES):
        i0 = I0[c]
        in_maps.append(
            {
                "xs": np.ascontiguousarray(x[:, i0 : i0 + NR, :]),
                "ks": np.ascontiguousarray(kernels[i0 : i0 + NI]),
                "bs": np.ascontiguousarray(bias[i0 : i0 + NI]),
            }
        )

    kw = dict(trace=True) if TRACE else {}
    res = run_bass_kernel_spmd(nc, in_maps, core_ids=list(range(NCORES)), **kw)
    global LAST_RESULTS
    LAST_RESULTS = res
    results = res.results

    out = np.empty((B, OH, OW), dtype=np.float32)
    for c in range(NCORES):
        i0 = I0[c]
        out[:, i0 : i0 + NI, :] = results[c]["os"]
    return out


# revision 14
# speedup vs baseline: 1.9975x; 1.9975x over previous
"""LocallyConnected2d (untied-weights conv) Trainium2 kernel.

Math: out[b, i, j] = sum_{kh, kw} x[b, i+kh, j+kw] * K[i, j, kh, kw] + bias[i, j]
with B=64, input 128x128, taps 8x8, output 121x121, fp32 in/out.

Strategy (output rows sharded across 8 cores, full batch per core):
  Per-output-position weights rule out weight-stationary matmul, but for a
  fixed output row i and tap row kh the contribution
    C[b, j] = sum_kw K[i, j, kh, kw] * x[b, i+kh, j+kw]
  is a matmul over the full input row (contraction w = j + kw):
    C[b, j] = sum_w xT_r[w, b] * Band_t[w, j],  r = i + kh,  t = i*8 + kh
  where Band_t[w, c=j+7] = K[i, j, kh, w-j] for w-j in [0, 8) — a banded
  128x128 matrix. PSUM (fp32) accumulates the 8 kh contributions per output
  row, initialized with the bias via a K=1 ones-matmul, so each output row
  costs 8 TensorEngine matmuls of N=121 columns. Matmul operands are bf16
  (values round-trip through bf16; accumulation stays fp32).

  Band construction avoids per-tile DMAs (HWDGE costs ~0.75us per DMA
  instruction) by building all band CONTENT lane-locally: partition t of
  RApad holds the full 128x128 band image of tile t (the skew is then a
  free-dim access pattern, written by one strided DVE copy + a 7x7 corner
  copy over a zeroed background). A 4-chunk DRAM bounce then performs the
  (t, w) block transpose: SBUF->DRAM with a w-major DRAM access pattern,
  DRAM->SBUF as plain loads into the w-partitioned band tensor BT2.

  xT (x rows transposed to [w, b] bf16) is produced by PE transposes.
"""

import sys

import numpy as np

try:
    import concourse.bacc as bacc
except ImportError:
    sys.path.insert(0, "/opt/trn_rl_repo")
    import concourse.bacc as bacc

import concourse.mybir as mybir
from concourse import tile
from concourse.bass_utils import run_bass_kernel_spmd
from concourse.masks import make_identity

F32 = mybir.dt.float32
BF16 = mybir.dt.bfloat16

B = 64
IN_H = IN_W = 128
KH = KW = 8
OH = OW = 121
NCORES = 8
NI = 16              # output rows computed per core
NR = NI + KH - 1     # input rows needed per core (23)
NT = NI * KH         # band tiles per core (128)
I0 = [0, 16, 32, 48, 64, 80, 96, 105]  # first output row per core (core 7 overlaps)

SLOT = 16                      # kw slots per j (8 real + 8 zero pad)
KT_COLS = OW + KH + 6          # 135 j-slots (j' = j + 7, zero pad both ends)
KT_ROW = KT_COLS * SLOT        # 2160 elements per partition
BROW = 128                     # band tile columns (c = j + 7)
NCHUNK = 4                     # bounce pipeline chunks (t-ranges)
TCH = NT // NCHUNK             # tiles per chunk (32)

_nc_cache = []
TRACE = False          # set by test.py to capture an NTFF profile
LAST_RESULTS = None    # BassKernelResults of the most recent run


def _build_nc():
    nc = bacc.Bacc(detect_race_conditions=False)
    x_in = nc.dram_tensor("xs", [B, NR, IN_W], F32, kind="ExternalInput")
    k_in = nc.dram_tensor("ks", [NI, OW, KH, KW], F32, kind="ExternalInput")
    b_in = nc.dram_tensor("bs", [NI, OW], F32, kind="ExternalInput")
    o_out = nc.dram_tensor("os", [B, NI, OW], F32, kind="ExternalOutput")

    with tile.TileContext(nc) as tc:
        with (
            tc.tile_pool(name="sb", bufs=1) as sb,
            tc.tile_pool(name="ps", bufs=1, space="PSUM") as ps,
            tc.tile_pool(name="dr", bufs=1, space="DRAM") as dr,
        ):
            XS = sb.tile([B, NR * IN_W], F32, tag="xs")       # x slab, b on partitions
            XT = sb.tile([128, NR * B], BF16, tag="xt")       # x rows transposed, bf16
            KT = sb.tile([128, KT_ROW], F32, tag="kt")        # taps, part=(ig,kh)
            RAP = sb.tile([128, 128 * BROW], BF16, tag="rap")  # band image, part=t
            BT2 = sb.tile([128, NT * BROW], BF16, tag="bt2")   # bands, part=w
            OS = sb.tile([B, NI * OW], F32, tag="os")          # output staging
            BIAS = sb.tile([1, NI * OW], F32, tag="bias")
            ONES = sb.tile([1, B], F32, tag="ones")
            ID64 = sb.tile([B, B], F32, tag="id64")
            BD = dr.tile([NCHUNK, 128 * TCH * BROW], BF16, tag="bd", name="bd")

            PSB = [
                ps.tile([B, 4 * OW], F32, tag=f"psb{g}", name=f"psb{g}")
                for g in range(4)
            ]

            # ---- input DMAs ----
            nc.sync.dma_start(out=XS[:, :], in_=x_in[:, :, :])
            nc.scalar.dma_start(out=BIAS[0:1, :], in_=b_in[:, :])
            # zero KT (covers j pads and kw slots 8..15), then load taps:
            # KT[ig*8+kh][(j+7)*16 + kw] = K[ig, j, kh, kw]; one DMA per kh,
            # split across the two HWDGE rings.
            nc.vector.memset(KT[:, 0 : KT_ROW // 2], 0.0)
            nc.gpsimd.memset(KT[:, KT_ROW // 2 : KT_ROW], 0.0)
            for kh in range(KH):
                kt_dst = KT[kh : kh + 1, 7 * SLOT : 7 * SLOT + 1].copy()
                kt_dst.ap[:] = [[KH * KT_ROW, NI], [SLOT, OW], [1, KW]]
                kt_src = k_in[0, 0, kh, 0:1].copy()  # dims (ig, j, kw)
                kt_src.ap[:] = [[OW * KH * KW, NI], [KH * KW, OW], [1, KW]]
                eng = nc.sync if kh % 2 == 0 else nc.scalar
                eng.dma_start(out=kt_dst, in_=kt_src)

            # ---- constants ----
            make_identity(nc, ID64[:, :])
            nc.gpsimd.memset(ONES[0:1, :], 1.0)

            # ---- RApad: per-partition t, the full band image (zeros + skew) ----
            nc.vector.memset(RAP[:, 0 : 64 * BROW], 0.0)
            nc.gpsimd.memset(RAP[:, 64 * BROW : 128 * BROW], 0.0)
            # main diagonal runs: RAP[t][129*w + q] = KT[t][(w+q)*16 + (7-q)]
            sk_dst = RAP[:, 0:1].copy()
            sk_dst.ap[:] = [[128 * BROW, 128], [BROW + 1, OW], [1, KW]]
            sk_src = KT[:, 7 : 7 + 1].copy()
            sk_src.ap[:] = [[KT_ROW, 128], [SLOT, OW], [SLOT - 1, KW]]
            nc.vector.tensor_copy(out=sk_dst, in_=sk_src)
            # 7x7 corner: RAP[t][128*(121+w') + 121+c'] = KT[t][(c'+121)*16 + (w'-c'+7)]
            co_dst = RAP[:, 121 * BROW + 121 : 121 * BROW + 121 + 1].copy()
            co_dst.ap[:] = [[128 * BROW, 128], [BROW, 7], [1, 7]]
            co_src = KT[:, 1943 : 1943 + 1].copy()
            co_src.ap[:] = [[KT_ROW, 128], [1, 7], [SLOT - 1, 7]]
            nc.vector.tensor_copy(out=co_dst, in_=co_src)

            # ---- bounce: block-transpose RAP (part=t) -> BT2 (part=w) ----
            for k in range(NCHUNK):
                # hop 1: SBUF -> DRAM, chunk-local w-major layout
                h1_dst = BD[k : k + 1, 0:1].copy()
                h1_dst.ap[:] = [[BROW, TCH], [TCH * BROW, 128], [1, BROW]]
                h1_src = RAP[k * TCH : (k + 1) * TCH, :]
                eng = nc.sync if k % 2 == 0 else nc.scalar
                eng.dma_start(out=h1_dst, in_=h1_src)
                # hop 2: DRAM -> SBUF plain load into BT2 column chunk
                h2_src = BD[k : k + 1, 0:1].copy()
                h2_src.ap[:] = [[TCH * BROW, 128], [BROW, TCH], [1, BROW]]
                h2_dst = BT2[:, k * TCH * BROW : (k + 1) * TCH * BROW]
                eng2 = nc.scalar if k % 2 == 0 else nc.sync
                eng2.dma_start(out=h2_dst, in_=h2_src)

            # ---- PE transposes: x rows -> XT[w, r*64+b] (cast to bf16) ----
            for r in range(NR):
                tp = ps.tile([128, B], F32, tag="tp", bufs=3, name=f"tp{r}")
                nc.tensor.transpose(
                    tp[:, :], XS[:, r * IN_W : (r + 1) * IN_W], ID64[:, :]
                )
                nc.scalar.copy(out=XT[:, r * B : (r + 1) * B], in_=tp[:, :])

            # ---- bias init (K=1 ones-matmul, whole bank, starts the group) ----
            for g in range(4):
                nc.tensor.matmul(
                    PSB[g][0:B, :],
                    ONES[0:1, :],
                    BIAS[0:1, g * 4 * OW : (g + 1) * 4 * OW],
                    start=True,
                    stop=False,
                    skip_group_check=True,
                )

            # ---- main banded matmuls (bf16 operands, fp32 accumulate) ----
            for rl in range(NR):
                lhsT = XT[:, rl * B : (rl + 1) * B]
                for kh in range(KH):
                    il = rl - kh
                    if 0 <= il < NI:
                        t = il * KH + kh
                        g, s = il // 4, il % 4
                        nc.tensor.matmul(
                            PSB[g][0:B, s * OW : (s + 1) * OW],
                            lhsT,
                            BT2[:, t * BROW + 7 : t * BROW + BROW],
                            start=False,
                            stop=(s == 3 and kh == KH - 1),
                            skip_group_check=True,
                        )
                # retire a full psum bank once its 4 output rows are done
                for g in range(4):
                    if rl == 4 * g + 3 + KH - 1:
                        nc.scalar.copy(
                            out=OS[:, g * 4 * OW : (g + 1) * 4 * OW],
                            in_=PSB[g][0:B, :],
                        )

            nc.sync.dma_start(out=o_out[:, :, :], in_=OS[:, :])

    nc.finalize()
    return nc


def kernel(x, kernels, bias):
    x = np.ascontiguousarray(x, dtype=np.float32)
    kernels = np.ascontiguousarray(kernels, dtype=np.float32)
    bias = np.ascontiguousarray(bias, dtype=np.float32)

    if not _nc_cache:
        _nc_cache.append(_build_nc())
    nc = _nc_cache[0]

    in_maps = []
    for c in range(NCORES):
        i0 = I0[c]
        in_maps.append(
            {
                "xs": np.ascontiguousarray(x[:, i0 : i0 + NR, :]),
                "ks": np.ascontiguousarray(kernels[i0 : i0 + NI]),
                "bs": np.ascontiguousarray(bias[i0 : i0 + NI]),
            }
        )

    kw = dict(trace=True) if TRACE else {}
    res = run_bass_kernel_spmd(nc, in_maps, core_ids=list(range(NCORES)), **kw)
    global LAST_RESULTS
    LAST_RESULTS = res
    results = res.results

    out = np.empty((B, OH, OW), dtype=np.float32)
    for c in range(NCORES):
        i0 = I0[c]
        out[:, i0 : i0 + NI, :] = results[c]["os"]
    return out
